# revision 35
# baseline (speedup 1.0000x reference)
"""Mamba-1 SSM block (LayerNorm -> in_proj -> causal conv -> selective scan
-> gated out_proj -> relu + residual) on 8 Trainium2 NeuronCores.

Sharding: core c handles batch b = c//2 and d_inner half h = c%2.
Each core computes the x-path (in_proj x part, conv, x_dbl) for ALL
d_inner channels (cheap duplication that avoids a mid-pipeline collective),
but runs delta/scan/gating only for its own 768 channels.  out_proj is
computed over own channels for all 1024 timesteps; a pair AllReduce sums
the two channel-half partials, after which both cores of a pair hold the
full output (the host keeps the even core's copy).

v2: the six 128-channel groups are fused into single wide tiles
([128, 6, L] activations, [128, 6, N, TC] scan state) so each scan-loop
step is one large DVE instruction instead of six small ones; dA
generation runs on the Activation engine as exp(-(n+1)*delta) scaled-Exp
ops; the scan runs in place (h overwrites dBu) to fit SBUF.

Channel order is permuted per-core to [own 768, peer 768] on the host so
the SPMD program can index "own half" statically.
"""

import numpy as np
import ml_dtypes
from contextlib import ExitStack

import concourse.bass as bass
import concourse.bacc as bacc
import concourse.tile as tile
from concourse import mybir
from concourse.bass_utils import run_bass_kernel_spmd
from concourse.masks import make_identity

F32 = mybir.dt.float32
BF16 = mybir.dt.bfloat16
NPBF16 = ml_dtypes.bfloat16
OP = mybir.AluOpType
AF = mybir.ActivationFunctionType

B, L, D = 4, 1024, 768
DI = 1536          # d_inner
DH = 768           # d_inner half per core
N = 16             # d_state
DCONV = 4
DTR = 48           # dt_rank
EPS = 1e-6
EPROJ = DI + DH    # in_proj output channels per core: full x + own z
TC = 128           # scan time-chunk
G = 6              # own-channel groups of 128
CFG = {"stop": "full", "part_bf16": True, "da_pow": True,
       "buckets": [2, 2, 2, 1, 1], "ep_delay": 4, "ln_evict": "vector", "tail_ep": "vector",
       "conv_tsm": False, "bc_bufs": 2, "skip_readout": False,
       "skip_gen": False, "skip_scan": False, "skip_coll": False,
       "skip_op": False}
NCH = L // TC


def ap_view(t, extra_off, ap_list):
    """Build a custom AP over an existing tile's storage."""
    return bass.AP(tensor=t.tensor, offset=t.offset + extra_off, ap=ap_list)


def build_program(cfg=None, repeat=1):
    CFG = dict(globals()["CFG"])
    if cfg:
        CFG.update(cfg)
    nc = bacc.Bacc(num_devices=8)

    x_in = nc.dram_tensor("x_in", [L, D], F32, kind="ExternalInput")
    w_in_t = nc.dram_tensor("w_in_t", [D, EPROJ], BF16, kind="ExternalInput")
    bias_in = nc.dram_tensor("bias_in", [EPROJ], F32, kind="ExternalInput")
    w_conv = nc.dram_tensor("w_conv", [DI, DCONV], F32, kind="ExternalInput")
    b_conv = nc.dram_tensor("b_conv", [DI], F32, kind="ExternalInput")
    w_x_t = nc.dram_tensor("w_x_t", [DI, DTR + 2 * N], BF16, kind="ExternalInput")
    w_dt_t = nc.dram_tensor("w_dt_t", [DTR, DH], BF16, kind="ExternalInput")
    b_dt = nc.dram_tensor("b_dt", [DH], F32, kind="ExternalInput")
    a_neg = nc.dram_tensor("a_neg", [DH, N], BF16, kind="ExternalInput")
    d_par = nc.dram_tensor("d_par", [DH], F32, kind="ExternalInput")
    w_out_t = nc.dram_tensor("w_out_t", [DH, D], BF16, kind="ExternalInput")
    out_d = nc.dram_tensor("out", [L, D], F32, kind="ExternalOutput")

    PDT = BF16 if CFG["part_bf16"] else F32
    BK = CFG["buckets"]        # chunks per AllReduce bucket (sums to NCH)
    assert sum(BK) == NCH
    NB = len(BK)
    bend = list(np.cumsum(BK))             # bucket end chunk (exclusive)
    bstart = [e - k for e, k in zip(bend, BK)]
    ch2b = [next(i for i in range(NB) if c < bend[i]) for c in range(NCH)]
    bounces = [
        (
            nc.dram_tensor(f"bc_bounce_r{r}", [2 * N, L], BF16),
            [nc.dram_tensor(f"part_d{i}_r{r}", [BK[i] * TC, D], PDT) for i in range(NB)],
            [nc.dram_tensor(f"sum_d{i}_r{r}", [BK[i] * TC, D], PDT) for i in range(NB)],
        )
        for r in range(repeat)
    ]

    with tile.TileContext(nc) as tc, ExitStack() as ctx:
        consts = ctx.enter_context(tc.tile_pool(name="consts", bufs=1))
        wpool = ctx.enter_context(tc.tile_pool(name="wpool", bufs=1))
        lnp = ctx.enter_context(tc.tile_pool(name="lnp", bufs=3))
        bigp = ctx.enter_context(tc.tile_pool(name="bigp", bufs=1))
        scanp = ctx.enter_context(tc.tile_pool(name="scanp", bufs=1))
        bccp = ctx.enter_context(tc.tile_pool(name="bccp", bufs=CFG["bc_bufs"]))
        rot = ctx.enter_context(tc.tile_pool(name="rot", bufs=2))
        outp = ctx.enter_context(tc.tile_pool(name="outp", bufs=2))
        psum = ctx.enter_context(tc.tile_pool(name="psum", bufs=6, space="PSUM"))
        pst = ctx.enter_context(tc.tile_pool(name="pst", bufs=2, space="PSUM"))

        # ---------------- constants ----------------
        ident = consts.tile([128, 128], BF16)
        make_identity(nc, ident)
        eps_t = consts.tile([128, 1], F32)
        nc.vector.memset(eps_t, EPS)
        wconv_t = consts.tile([128, 12, DCONV], F32)
        nc.sync.dma_start(out=wconv_t, in_=w_conv[:].rearrange("(g p) k -> p g k", p=128))
        bconv_t = consts.tile([128, 12], F32)
        nc.sync.dma_start(out=bconv_t, in_=b_conv[:].rearrange("(g p) -> p g", p=128))
        bdt_t = consts.tile([128, G], F32)
        nc.sync.dma_start(out=bdt_t, in_=b_dt[:].rearrange("(g p) -> p g", p=128))
        dpar_t = consts.tile([128, G], F32)
        nc.sync.dma_start(out=dpar_t, in_=d_par[:].rearrange("(g p) -> p g", p=128))
        a_t = consts.tile([128, G, N], BF16)
        nc.sync.dma_start(out=a_t, in_=a_neg[:].rearrange("(g p) n -> p g n", p=128))
        biasin_t = consts.tile([128, 18], F32)
        nc.sync.dma_start(out=biasin_t, in_=bias_in[:].rearrange("(m p) -> p m", p=128))

        # ---------------- weights (loaded once; nothing overwrites them).
        # DMAs are emitted inside body(0) AFTER the LayerNorm x loads so the
        # first chunk's input path isn't stuck behind 5MB of weights on the
        # DMA queue.
        w_dt_sb = wpool.tile([DTR, DH], BF16, tag="w_dt")
        w_in_sb = [wpool.tile([128, EPROJ], BF16, tag=f"w_in{k}", name=f"w_in{k}") for k in range(6)]
        w_x_sb = [wpool.tile([128, DTR + 2 * N], BF16, tag=f"w_x{k}", name=f"w_x{k}") for k in range(12)]
        w_out_sb = [wpool.tile([128, D], BF16, tag=f"w_out{k}", name=f"w_out{k}") for k in range(6)]

        def load_weights():
            for k in range(6):
                nc.sync.dma_start(out=w_in_sb[k], in_=w_in_t[k * 128:(k + 1) * 128, :])
            for k in range(12):
                nc.sync.dma_start(out=w_x_sb[k], in_=w_x_t[k * 128:(k + 1) * 128, :])
            nc.sync.dma_start(out=w_dt_sb, in_=w_dt_t[:])
            for k in range(6):
                nc.sync.dma_start(out=w_out_sb[k], in_=w_out_t[k * 128:(k + 1) * 128, :])

        # ---------------- persistent fused tiles ----------------
        # xnys: holds xn^T during LN/in_proj, then is reused as ys.
        HL = L // 2  # stage-pipeline half length
        xnys = bigp.tile([128, G, L], BF16, tag="xnys")
        # x-path tiles, one per time half: [3-col causal edge | 512 cols].
        # Groups 0..5 = own channels (conv output in place); groups 6..11
        # hold peer channels through x_dbl, then are reused for E=D*u*silu(z).
        xph = [bigp.tile([128, 12, HL + 3], BF16, tag=f"xp{h}", name=f"xp{h}")
               for h in range(2)]
        sz = bigp.tile([128, G, L], BF16, tag="sz")
        delta = bigp.tile([128, G, L], BF16, tag="delta")
        dt_t = bigp.tile([DTR, L], BF16, tag="dt_t")
        bc_sb = bigp.tile([2 * N, L], BF16, tag="bc_sb")
        carry = bigp.tile([128, G, N], BF16, tag="carry")
        da = scanp.tile([128, G, N, TC], BF16, tag="da")
        dbu = scanp.tile([128, G, N, TC], BF16, tag="dbu")
        da_flat = da.rearrange("p g n t -> p (g n t)")
        dbu_flat = dbu.rearrange("p g n t -> p (g n t)")

        def body(rep):
            bc_bounce, part_ds, sum_ds = bounces[rep]
            ys = xnys  # reuse: xn^T is dead after in_proj reads it
            ep_next = [0]

            # ---------------- stage helpers (per time half ns) --------------
            def layer_norm(tiles):
                for tt in tiles:
                    xt = lnp.tile([128, D], F32, tag="xt", bufs=2)
                    nc.sync.dma_start(out=xt, in_=x_in[tt * 128:(tt + 1) * 128, :])
                    stats = lnp.tile([128, 3, 6], F32, tag="stats")
                    for s in range(3):
                        nc.vector.bn_stats(out=stats[:, s, :], in_=xt[:, s * 256:(s + 1) * 256])
                    mv = lnp.tile([128, 2], F32, tag="mv")
                    nc.vector.bn_aggr(out=mv, in_=stats)
                    sd = lnp.tile([128, 1], F32, tag="sd")
                    nc.scalar.activation(out=sd, in_=mv[:, 1:2], func=AF.Sqrt, bias=eps_t)
                    rs = lnp.tile([128, 1], F32, tag="rs")
                    nc.vector.reciprocal(out=rs, in_=sd)
                    xnb = lnp.tile([128, D], BF16, tag="xnb", bufs=2)
                    nc.vector.tensor_scalar(
                        out=xnb, in0=xt, scalar1=mv[:, 0:1], scalar2=rs,
                        op0=OP.subtract, op1=OP.mult)
                    for dd in range(6):
                        ps = pst.tile([128, 128], BF16, tag="ps_t")
                        nc.tensor.transpose(ps, xnb[:, dd * 128:(dd + 1) * 128], ident)
                        if CFG["ln_evict"] == "vector":
                            nc.vector.tensor_copy(
                                out=xnys[:, dd, tt * 128:(tt + 1) * 128], in_=ps)
                        else:
                            nc.scalar.copy(
                                out=xnys[:, dd, tt * 128:(tt + 1) * 128], in_=ps)

            def in_proj_half(ns):
                # m-tiles 0..11 -> x (local order [own, peer]); 12..17 -> z own
                for m in range(18):
                    ps = psum.tile([128, 512], F32, tag="ps_mm")
                    for k in range(6):
                        nc.tensor.matmul(
                            ps, w_in_sb[k][:, m * 128:(m + 1) * 128],
                            xnys[:, k, ns * HL:(ns + 1) * HL],
                            start=(k == 0), stop=(k == 5))
                    if m < 12:
                        nc.scalar.activation(
                            out=xph[ns][:, m, 3:3 + HL], in_=ps,
                            func=AF.Identity, bias=biasin_t[:, m:m + 1])
                    else:
                        nc.scalar.activation(
                            out=sz[:, m - 12, ns * HL:(ns + 1) * HL], in_=ps,
                            func=AF.Silu, bias=biasin_t[:, m:m + 1])

            def conv_half(ns):
                # causal depthwise conv + silu, output in place over the input.
                # Edge cols 0:3 of half 1 are pre-conv copies from half 0.
                if ns == 0:
                    nc.vector.memset(xph[0][:, :, 0:3], 0.0)
                    nc.vector.tensor_copy(
                        out=xph[1][:, :, 0:3], in_=xph[0][:, :, HL:HL + 3])
                for g in range(12):
                    acc = rot.tile([128, HL], BF16, tag="conv_acc", bufs=2)
                    if CFG["conv_tsm"]:
                        # 4 tensor_scalar muls (4x DVE mode) + tree of adds
                        acb = rot.tile([128, HL], BF16, tag="conv_acb", bufs=2)
                        nc.vector.tensor_scalar_mul(acc, xph[ns][:, g, 0:HL], wconv_t[:, g, 0:1])
                        nc.vector.tensor_scalar_mul(acb, xph[ns][:, g, 1:1 + HL], wconv_t[:, g, 1:2])
                        nc.vector.tensor_add(acc, acc, acb)
                        nc.vector.tensor_scalar_mul(acb, xph[ns][:, g, 2:2 + HL], wconv_t[:, g, 2:3])
                        nc.vector.tensor_add(acc, acc, acb)
                        nc.vector.tensor_scalar_mul(acb, xph[ns][:, g, 3:3 + HL], wconv_t[:, g, 3:4])
                        nc.vector.tensor_add(acc, acc, acb)
                    else:
                        nc.vector.tensor_scalar_mul(acc, xph[ns][:, g, 0:HL], wconv_t[:, g, 0:1])
                        for k in range(1, 4):
                            nc.vector.scalar_tensor_tensor(
                                out=acc, in0=xph[ns][:, g, k:k + HL],
                                scalar=wconv_t[:, g, k:k + 1],
                                in1=acc, op0=OP.mult, op1=OP.add)
                    nc.scalar.activation(
                        out=xph[ns][:, g, 3:3 + HL], in_=acc, func=AF.Silu,
                        bias=bconv_t[:, g:g + 1])

            def xdbl_half(ns):
                ps = psum.tile([128, 512], F32, tag="ps_mm")
                for k in range(12):
                    nc.tensor.matmul(
                        ps[0:DTR + 2 * N, :], w_x_sb[k],
                        xph[ns][:, k, 3:3 + HL],
                        start=(k == 0), stop=(k == 11))
                nc.vector.tensor_copy(
                    out=bc_sb[:, ns * HL:(ns + 1) * HL], in_=ps[0:2 * N, :])
                nc.scalar.copy(
                    out=dt_t[0:32, ns * HL:(ns + 1) * HL], in_=ps[32:64, :])
                nc.scalar.copy(
                    out=dt_t[32:DTR, ns * HL:(ns + 1) * HL], in_=ps[64:2 * N + DTR, :])
                nc.sync.dma_start(
                    out=bc_bounce[:, ns * HL:(ns + 1) * HL],
                    in_=bc_sb[:, ns * HL:(ns + 1) * HL])

            def delta_half(ns):
                # delta = softplus(W_dt^T @ dt + b_dt): per-m Exp into delta,
                # then one fused Ln(1+e^x) in place (2 ACT table swaps total).
                pss = []
                for m in range(G):
                    ps = psum.tile([128, 512], F32, tag="ps_mm", name=f"psd{m}")
                    nc.tensor.matmul(
                        ps, w_dt_sb[:, m * 128:(m + 1) * 128],
                        dt_t[:, ns * HL:(ns + 1) * HL], start=True, stop=True)
                    pss.append(ps)
                for m in range(G):
                    nc.scalar.activation(
                        out=delta[:, m, ns * HL:(ns + 1) * HL], in_=pss[m],
                        func=AF.Exp, bias=bdt_t[:, m:m + 1])
                dsl = delta[:, :, ns * HL:(ns + 1) * HL]
                nc.scalar.activation(out=dsl, in_=dsl, func=AF.Ln, bias=1.0)

            def e_half(ns):
                # E = D*u*silu(z) into the dead peer half of xph[ns]
                for g in range(G):
                    nc.vector.scalar_tensor_tensor(
                        out=xph[ns][:, 6 + g, 3:3 + HL],
                        in0=xph[ns][:, g, 3:3 + HL], scalar=dpar_t[:, g:g + 1],
                        in1=sz[:, g, ns * HL:(ns + 1) * HL],
                        op0=OP.mult, op1=OP.mult)

            def epilogue(m, tail=False):
                # relu + residual + store for time tile m (after its AllReduce).
                # Tail epilogues (emitted after the chunk loop) run entirely on
                # the gpsimd/Pool queue: they wait on the last collectives, and
                # doing that on DVE/SP would head-block the next rep's work.
                tail = tail and CFG["tail_ep"] != "vector"
                eng = nc.gpsimd if tail else nc.vector
                dma = nc.gpsimd if (tail and CFG["tail_ep"] == "gpsimd") else nc.sync
                i = ch2b[m]
                s_sb = outp.tile([128, D], PDT, tag="s_sb")
                dma.dma_start(
                    out=s_sb,
                    in_=sum_ds[i][(m - bstart[i]) * TC:(m - bstart[i] + 1) * TC, :])
                xres = outp.tile([128, D], F32, tag="xres")
                dma.dma_start(out=xres, in_=x_in[m * 128:(m + 1) * 128, :])
                o2 = outp.tile([128, D], F32, tag="o2", bufs=2)
                eng.tensor_scalar_max(o2, s_sb, 0.0)
                eng.tensor_add(o2, o2, xres)
                dma.dma_start(out=out_d[m * 128:(m + 1) * 128, :], in_=o2)

            def gen_da(c):
                # dA[:, g, n, :] = exp(-(n+1) * delta) on the ACT engine.
                # Emitted right after scan(c-1) consumes da so the ACT work
                # overlaps the DVE readout of the previous chunk.
                d_sl = delta[:, :, c * TC:(c + 1) * TC]
                if CFG["da_pow"]:
                    for n in range(N):
                        nc.scalar.activation(
                            out=da[:, :, n, :], in_=d_sl,
                            func=AF.Exp, scale=-(n + 1.0))
                else:
                    nc.vector.tensor_mul(
                        da, d_sl[:, :, None, :].broadcast_to([128, G, N, TC]),
                        a_t[:, :, :, None].broadcast_to([128, G, N, TC]))
                    nc.scalar.activation(out=da_flat, in_=da_flat, func=AF.Exp)

            def chunk(c):
                csl = slice(c * TC, (c + 1) * TC)
                h, lo = c // (NCH // 2), (c % (NCH // 2)) * TC
                # one broadcast DMA for both B (rows 0:N) and C (rows N:2N)
                bcc = bccp.tile([128, 2 * N, TC], BF16, tag="bcc")
                nc.sync.dma_start(
                    out=bcc,
                    in_=ap_view(bc_bounce[:], c * TC, [[0, 128], [L, 2 * N], [1, TC]]))
                d_sl = delta[:, :, csl]
                u_sl = xph[h][:, 0:G, 3 + lo:3 + lo + TC]
                if not CFG["skip_gen"]:
                    # du = delta * u ; dBu[:, g, n, :] = du (bcast n) * B (bcast g)
                    du = rot.tile([128, G, TC], BF16, tag="du", bufs=2)
                    nc.vector.tensor_mul(du, d_sl, u_sl)
                    nc.vector.tensor_mul(
                        dbu, du[:, :, None, :].broadcast_to([128, G, N, TC]),
                        bcc[:, None, 0:N, :].broadcast_to([128, G, N, TC]))
                # fold carry into the t=0 column: h[n,0] = dA0*carry + dBu0
                if c > 0:
                    ctmp = rot.tile([128, G, N], BF16, tag="ctmp", bufs=2)
                    nc.vector.tensor_mul(ctmp, carry, da[:, :, :, 0])
                    nc.vector.tensor_add(dbu[:, :, :, 0], dbu[:, :, :, 0], ctmp)
                nc.vector.memset(da[:, :, :, 0:1], 0.0)
                # scan along flattened (g, n, t), in place over dBu; dA=0 at
                # segment starts so states never cross (g, n) boundaries.
                if not CFG["skip_scan"]:
                    nc.vector.tensor_tensor_scan(
                        out=dbu_flat, data0=da_flat, data1=dbu_flat,
                        initial=0.0, op0=OP.mult, op1=OP.add)
                # da is free after the scan: generate the next chunk's dA now
                # (chunk 3's successor needs delta half 1, emitted before it)
                if c + 1 < NCH and not CFG["skip_gen"]:
                    gen_da(c + 1)
                nc.vector.tensor_copy(out=carry, in_=dbu[:, :, :, TC - 1])
                ys_sl = ys[:, :, csl]
                if CFG["skip_readout"]:
                    nc.vector.tensor_copy(out=ys_sl, in_=dbu[:, :, 0, :])
                else:
                    # readout: hC in place -> tree-reduce over n -> ys
                    nc.vector.tensor_mul(
                        dbu, dbu, bcc[:, None, N:2 * N, :].broadcast_to([128, G, N, TC]))
                    nc.vector.tensor_add(dbu[:, :, 0:8, :], dbu[:, :, 0:8, :], dbu[:, :, 8:16, :])
                    nc.vector.tensor_add(dbu[:, :, 8:12, :], dbu[:, :, 0:4, :], dbu[:, :, 4:8, :])
                    nc.vector.tensor_add(dbu[:, :, 12:14, :], dbu[:, :, 8:10, :], dbu[:, :, 10:12, :])
                    nc.vector.tensor_add(ys_sl, dbu[:, :, 12, :], dbu[:, :, 13, :])
                    # gate: y = ys*silu(z) + D*u*silu(z)
                    nc.vector.tensor_mul(ys_sl, ys_sl, sz[:, :, csl])
                    nc.vector.tensor_add(
                        ys_sl, ys_sl, xph[h][:, 6:12, 3 + lo:3 + lo + TC])
                # out_proj m-tile for this chunk's time columns (m == c)
                if CFG["skip_op"]:
                    return
                m = c
                part = outp.tile([128, D], PDT, tag="part")
                for ns, nw in ((0, 512), (1, 256)):
                    ps = psum.tile([128, 512], F32, tag="ps_mm")
                    for k in range(6):
                        nc.tensor.matmul(
                            ps[:, 0:nw], ys[:, k, m * 128:(m + 1) * 128],
                            w_out_sb[k][:, ns * 512:ns * 512 + nw],
                            start=(k == 0), stop=(k == 5))
                    nc.scalar.copy(
                        out=part[:, ns * 512:ns * 512 + nw], in_=ps[:, 0:nw])
                if CFG["skip_coll"]:
                    return
                i = ch2b[m]
                nc.sync.dma_start(
                    out=part_ds[i][(m - bstart[i]) * TC:(m - bstart[i] + 1) * TC, :],
                    in_=part)
                # pair AllReduce per bucket; epilogues delayed so the
                # collective latency hides behind later scan work
                if m == bend[i] - 1:
                    nc.gpsimd.collective_compute(
                        "AllReduce", OP.add,
                        replica_groups=[[0, 1], [2, 3], [4, 5], [6, 7]],
                        ins=[part_ds[i][:]],
                        outs=[sum_ds[i][:]])
                lim = c + 1 - CFG["ep_delay"]
                allowed = sum(k for e, k in zip(bend, BK) if e <= lim)
                while ep_next[0] < min(allowed, NCH):
                    epilogue(ep_next[0])
                    ep_next[0] += 1

            # ------------- pipelined schedule: half-1 stages overlap -------
            # the scan of chunks 0..2 (PE/ACT work hides under DVE).
            # LN tiles 4-7 are deferred so in_proj half 0 (which only needs
            # xn cols 0:512 = tiles 0-3) starts as early as possible.
            layer_norm([0, 1, 2, 3])
            if rep == 0:
                load_weights()
            in_proj_half(0)
            conv_half(0)
            xdbl_half(0)
            if CFG["stop"] == "conv":
                layer_norm([4, 5, 6, 7])
                in_proj_half(1); conv_half(1)
                return
            delta_half(0)
            e_half(0)
            if CFG["stop"] == "xdbl":
                layer_norm([4, 5, 6, 7])
                in_proj_half(1); conv_half(1); xdbl_half(1)
                return
            if CFG["stop"] == "delta":
                layer_norm([4, 5, 6, 7])
                in_proj_half(1); conv_half(1); xdbl_half(1)
                delta_half(1); e_half(1)
                return
            if not CFG["skip_gen"]:
                gen_da(0)
            layer_norm([4, 5, 6, 7])
            chunk(0)
            in_proj_half(1)
            chunk(1)
            chunk(2)
            conv_half(1)
            xdbl_half(1)
            delta_half(1)
            e_half(1)
            for c in range(3, NCH):
                chunk(c)

            if CFG["stop"] in ("scan", "outproj") or CFG["skip_coll"] or CFG["skip_op"]:
                return
            # ---------------- remaining epilogues (Pool queue) ----------------
            while ep_next[0] < NCH:
                epilogue(ep_next[0], tail=True)
                ep_next[0] += 1

        for rep in range(repeat):
            body(rep)

    nc.compile()
    return nc


def make_in_maps(inputs):
    """Host-side sharding: per-core input dicts."""
    x = np.asarray(inputs["input_data"], np.float32)
    ln_g = np.asarray(inputs["ln_g"], np.float32)
    ln_b = np.asarray(inputs["ln_b"], np.float32)
    W_in = np.asarray(inputs["W_in"], np.float32)
    W_conv = np.asarray(inputs["W_conv"], np.float32)
    b_conv = np.asarray(inputs["b_conv"], np.float32)
    W_x = np.asarray(inputs["W_x"], np.float32)
    W_dt = np.asarray(inputs["W_dt"], np.float32)
    b_dt = np.asarray(inputs["b_dt"], np.float32)
    A_log = np.asarray(inputs["A_log"], np.float32)
    D_param = np.asarray(inputs["D_param"], np.float32)
    W_out = np.asarray(inputs["W_out"], np.float32)

    maps = []
    for c in range(8):
        b, h = c // 2, c % 2
        own = np.arange(h * DH, (h + 1) * DH)
        peer = np.arange((1 - h) * DH, (2 - h) * DH)
        perm = np.concatenate([own, peer])
        W_sel = np.concatenate([W_in[perm], W_in[DI + own]], 0)  # (2304, 768)
        maps.append({
            "x_in": np.ascontiguousarray(x[b]),
            "w_in_t": np.ascontiguousarray((W_sel * ln_g[None, :]).T).astype(NPBF16),
            "bias_in": np.ascontiguousarray(W_sel @ ln_b),
            "w_conv": np.ascontiguousarray(W_conv[perm, 0, :]),
            "b_conv": np.ascontiguousarray(b_conv[perm]),
            "w_x_t": np.ascontiguousarray(W_x[np.r_[DTR:DTR + 2 * N, 0:DTR]][:, perm].T).astype(NPBF16),
            "w_dt_t": np.ascontiguousarray(W_dt[own].T).astype(NPBF16),
            "b_dt": np.ascontiguousarray(b_dt[own]),
            "a_neg": np.ascontiguousarray(-np.exp(A_log[own])).astype(NPBF16),
            "d_par": np.ascontiguousarray(D_param[own]),
            "w_out_t": np.ascontiguousarray(W_out[:, own].T).astype(NPBF16),
        })
    return maps


_CACHED = {}


def kernel(**inputs) -> np.ndarray:
    # The fast dA path exploits A = -exp(A_log) = -[1..N] broadcast over
    # channels (dA_n = r^(n+1), r = exp(-delta)). Verify and fall back to
    # the general elementwise-exp path if A_log doesn't match that pattern.
    a_log = np.asarray(inputs["A_log"], np.float32)
    geo = np.log(np.broadcast_to(np.arange(1, N + 1, dtype=np.float32), (DI, N)))
    da_pow = bool(np.allclose(a_log, geo, rtol=1e-6, atol=1e-6))
    if da_pow not in _CACHED:
        _CACHED[da_pow] = build_program({"da_pow": da_pow})
    nc = _CACHED[da_pow]
    maps = make_in_maps(inputs)
    res = run_bass_kernel_spmd(nc, maps, core_ids=list(range(8)))
    out = np.stack([res.results[2 * b]["out"] for b in range(B)], 0)
    return out.astype(np.float32)


if __name__ == "__main__":
    rng = np.random.default_rng(0)
    ins = {
        "input_data": rng.standard_normal((B, L, D)).astype(np.float32),
        "ln_g": np.ones((D,), np.float32),
        "ln_b": np.zeros((D,), np.float32),
        "W_in": (rng.standard_normal((2 * DI, D)) * 0.02).astype(np.float32),
        "W_conv": (rng.standard_normal((DI, 1, DCONV)) * 0.02).astype(np.float32),
        "b_conv": np.zeros((DI,), np.float32),
        "W_x": (rng.standard_normal((DTR + 2 * N, DI)) * 0.02).astype(np.float32),
        "W_dt": (rng.standard_normal((DI, DTR)) * 0.02).astype(np.float32),
        "b_dt": (rng.standard_normal((DI,)) * 0.1).astype(np.float32),
        "A_log": np.log(np.broadcast_to(np.arange(1, N + 1, dtype=np.float32), (DI, N))).copy(),
        "D_param": np.ones((DI,), np.float32),
        "W_out": (rng.standard_normal((D, DI)) * 0.02).astype(np.float32),
    }
    out = kernel(**ins)
    print("kernel out", out.shape, out.dtype)


# revision 37
# speedup vs baseline: 13.8000x; 13.8000x over previous
"""Mamba-1 SSM block (LayerNorm -> in_proj -> causal conv -> selective scan
-> gated out_proj -> relu + residual) on 8 Trainium2 NeuronCores.

Sharding: core c handles batch b = c//2 and d_inner half h = c%2.
Each core computes the x-path (in_proj x part, conv, x_dbl) for ALL
d_inner channels (cheap duplication that avoids a mid-pipeline collective),
but runs delta/scan/gating only for its own 768 channels.  out_proj is
computed over own channels for all 1024 timesteps; a pair AllReduce sums
the two channel-half partials, after which both cores of a pair hold the
full output (the host keeps the even core's copy).

v2: the six 128-channel groups are fused into single wide tiles
([128, 6, L] activations, [128, 6, N, TC] scan state) so each scan-loop
step is one large DVE instruction instead of six small ones; dA
generation runs on the Activation engine as exp(-(n+1)*delta) scaled-Exp
ops; the scan runs in place (h overwrites dBu) to fit SBUF.

Channel order is permuted per-core to [own 768, peer 768] on the host so
the SPMD program can index "own half" statically.
"""

import numpy as np
import ml_dtypes
from contextlib import ExitStack

import concourse.bass as bass
import concourse.bacc as bacc
import concourse.tile as tile
from concourse import mybir
from concourse.bass_utils import run_bass_kernel_spmd
from concourse.masks import make_identity

F32 = mybir.dt.float32
BF16 = mybir.dt.bfloat16
NPBF16 = ml_dtypes.bfloat16
OP = mybir.AluOpType
AF = mybir.ActivationFunctionType

B, L, D = 4, 1024, 768
DI = 1536          # d_inner
DH = 768           # d_inner half per core
N = 16             # d_state
DCONV = 4
DTR = 48           # dt_rank
EPS = 1e-6
EPROJ = DI + DH    # in_proj output channels per core: full x + own z
TC = 128           # scan time-chunk
G = 6              # own-channel groups of 128
CFG = {"stop": "full", "part_bf16": True, "da_pow": True,
       "buckets": [2, 2, 2, 1, 1], "ep_delay": 4, "ln_evict": "vector", "tail_ep": "vector",
       "conv_tsm": False, "bc_bufs": 2, "ln_late": False, "skip_readout": False,
       "skip_gen": False, "skip_scan": False, "skip_coll": False,
       "skip_op": False}
NCH = L // TC


def ap_view(t, extra_off, ap_list):
    """Build a custom AP over an existing tile's storage."""
    return bass.AP(tensor=t.tensor, offset=t.offset + extra_off, ap=ap_list)


def build_program(cfg=None, repeat=1):
    CFG = dict(globals()["CFG"])
    if cfg:
        CFG.update(cfg)
    nc = bacc.Bacc(num_devices=8)

    x_in = nc.dram_tensor("x_in", [L, D], F32, kind="ExternalInput")
    w_in_t = nc.dram_tensor("w_in_t", [D, EPROJ], BF16, kind="ExternalInput")
    bias_in = nc.dram_tensor("bias_in", [EPROJ], F32, kind="ExternalInput")
    w_conv = nc.dram_tensor("w_conv", [DI, DCONV], F32, kind="ExternalInput")
    b_conv = nc.dram_tensor("b_conv", [DI], F32, kind="ExternalInput")
    w_x_t = nc.dram_tensor("w_x_t", [DI, DTR + 2 * N], BF16, kind="ExternalInput")
    w_dt_t = nc.dram_tensor("w_dt_t", [DTR, DH], BF16, kind="ExternalInput")
    b_dt = nc.dram_tensor("b_dt", [DH], F32, kind="ExternalInput")
    a_neg = nc.dram_tensor("a_neg", [DH, N], BF16, kind="ExternalInput")
    d_par = nc.dram_tensor("d_par", [DH], F32, kind="ExternalInput")
    w_out_t = nc.dram_tensor("w_out_t", [DH, D], BF16, kind="ExternalInput")
    out_d = nc.dram_tensor("out", [L, D], F32, kind="ExternalOutput")

    PDT = BF16 if CFG["part_bf16"] else F32
    BK = CFG["buckets"]        # chunks per AllReduce bucket (sums to NCH)
    assert sum(BK) == NCH
    NB = len(BK)
    bend = list(np.cumsum(BK))             # bucket end chunk (exclusive)
    bstart = [e - k for e, k in zip(bend, BK)]
    ch2b = [next(i for i in range(NB) if c < bend[i]) for c in range(NCH)]
    bounces = [
        (
            nc.dram_tensor(f"bc_bounce_r{r}", [2 * N, L], BF16),
            [nc.dram_tensor(f"part_d{i}_r{r}", [BK[i] * TC, D], PDT) for i in range(NB)],
            [nc.dram_tensor(f"sum_d{i}_r{r}", [BK[i] * TC, D], PDT) for i in range(NB)],
        )
        for r in range(repeat)
    ]

    with tile.TileContext(nc) as tc, ExitStack() as ctx:
        consts = ctx.enter_context(tc.tile_pool(name="consts", bufs=1))
        wpool = ctx.enter_context(tc.tile_pool(name="wpool", bufs=1))
        lnp = ctx.enter_context(tc.tile_pool(name="lnp", bufs=3))
        bigp = ctx.enter_context(tc.tile_pool(name="bigp", bufs=1))
        scanp = ctx.enter_context(tc.tile_pool(name="scanp", bufs=1))
        bccp = ctx.enter_context(tc.tile_pool(name="bccp", bufs=CFG["bc_bufs"]))
        rot = ctx.enter_context(tc.tile_pool(name="rot", bufs=2))
        outp = ctx.enter_context(tc.tile_pool(name="outp", bufs=2))
        psum = ctx.enter_context(tc.tile_pool(name="psum", bufs=6, space="PSUM"))
        pst = ctx.enter_context(tc.tile_pool(name="pst", bufs=2, space="PSUM"))

        # ---------------- constants ----------------
        ident = consts.tile([128, 128], BF16)
        make_identity(nc, ident)
        eps_t = consts.tile([128, 1], F32)
        nc.vector.memset(eps_t, EPS)
        wconv_t = consts.tile([128, 12, DCONV], F32)
        nc.sync.dma_start(out=wconv_t, in_=w_conv[:].rearrange("(g p) k -> p g k", p=128))
        bconv_t = consts.tile([128, 12], F32)
        nc.sync.dma_start(out=bconv_t, in_=b_conv[:].rearrange("(g p) -> p g", p=128))
        bdt_t = consts.tile([128, G], F32)
        nc.sync.dma_start(out=bdt_t, in_=b_dt[:].rearrange("(g p) -> p g", p=128))
        dpar_t = consts.tile([128, G], F32)
        nc.sync.dma_start(out=dpar_t, in_=d_par[:].rearrange("(g p) -> p g", p=128))
        a_t = consts.tile([128, G, N], BF16)
        nc.sync.dma_start(out=a_t, in_=a_neg[:].rearrange("(g p) n -> p g n", p=128))
        biasin_t = consts.tile([128, 18], F32)
        nc.sync.dma_start(out=biasin_t, in_=bias_in[:].rearrange("(m p) -> p m", p=128))

        # ---------------- weights (loaded once; nothing overwrites them).
        # DMAs are emitted inside body(0) AFTER the LayerNorm x loads so the
        # first chunk's input path isn't stuck behind 5MB of weights on the
        # DMA queue.
        w_dt_sb = wpool.tile([DTR, DH], BF16, tag="w_dt")
        w_in_sb = [wpool.tile([128, EPROJ], BF16, tag=f"w_in{k}", name=f"w_in{k}") for k in range(6)]
        w_x_sb = [wpool.tile([128, DTR + 2 * N], BF16, tag=f"w_x{k}", name=f"w_x{k}") for k in range(12)]
        w_out_sb = [wpool.tile([128, D], BF16, tag=f"w_out{k}", name=f"w_out{k}") for k in range(6)]

        def load_weights():
            for k in range(6):
                nc.sync.dma_start(out=w_in_sb[k], in_=w_in_t[k * 128:(k + 1) * 128, :])
            for k in range(12):
                nc.sync.dma_start(out=w_x_sb[k], in_=w_x_t[k * 128:(k + 1) * 128, :])
            nc.sync.dma_start(out=w_dt_sb, in_=w_dt_t[:])
            for k in range(6):
                nc.sync.dma_start(out=w_out_sb[k], in_=w_out_t[k * 128:(k + 1) * 128, :])

        # ---------------- persistent fused tiles ----------------
        # xnys: holds xn^T during LN/in_proj, then is reused as ys.
        HL = L // 2  # stage-pipeline half length
        xnys = bigp.tile([128, G, L], BF16, tag="xnys")
        # x-path tiles, one per time half: [3-col causal edge | 512 cols].
        # Groups 0..5 = own channels (conv output in place); groups 6..11
        # hold peer channels through x_dbl, then are reused for E=D*u*silu(z).
        xph = [bigp.tile([128, 12, HL + 3], BF16, tag=f"xp{h}", name=f"xp{h}")
               for h in range(2)]
        sz = bigp.tile([128, G, L], BF16, tag="sz")
        delta = bigp.tile([128, G, L], BF16, tag="delta")
        dt_t = bigp.tile([DTR, L], BF16, tag="dt_t")
        bc_sb = bigp.tile([2 * N, L], BF16, tag="bc_sb")
        carry = bigp.tile([128, G, N], BF16, tag="carry")
        da = scanp.tile([128, G, N, TC], BF16, tag="da")
        dbu = scanp.tile([128, G, N, TC], BF16, tag="dbu")
        da_flat = da.rearrange("p g n t -> p (g n t)")
        dbu_flat = dbu.rearrange("p g n t -> p (g n t)")

        def body(rep):
            bc_bounce, part_ds, sum_ds = bounces[rep]
            ys = xnys  # reuse: xn^T is dead after in_proj reads it
            ep_next = [0]

            # ---------------- stage helpers (per time half ns) --------------
            def layer_norm(tiles):
                for tt in tiles:
                    xt = lnp.tile([128, D], F32, tag="xt", bufs=2)
                    nc.sync.dma_start(out=xt, in_=x_in[tt * 128:(tt + 1) * 128, :])
                    stats = lnp.tile([128, 3, 6], F32, tag="stats")
                    for s in range(3):
                        nc.vector.bn_stats(out=stats[:, s, :], in_=xt[:, s * 256:(s + 1) * 256])
                    mv = lnp.tile([128, 2], F32, tag="mv")
                    nc.vector.bn_aggr(out=mv, in_=stats)
                    sd = lnp.tile([128, 1], F32, tag="sd")
                    nc.scalar.activation(out=sd, in_=mv[:, 1:2], func=AF.Sqrt, bias=eps_t)
                    rs = lnp.tile([128, 1], F32, tag="rs")
                    nc.vector.reciprocal(out=rs, in_=sd)
                    xnb = lnp.tile([128, D], BF16, tag="xnb", bufs=2)
                    nc.vector.tensor_scalar(
                        out=xnb, in0=xt, scalar1=mv[:, 0:1], scalar2=rs,
                        op0=OP.subtract, op1=OP.mult)
                    for dd in range(6):
                        ps = pst.tile([128, 128], BF16, tag="ps_t")
                        nc.tensor.transpose(ps, xnb[:, dd * 128:(dd + 1) * 128], ident)
                        if CFG["ln_evict"] == "vector":
                            nc.vector.tensor_copy(
                                out=xnys[:, dd, tt * 128:(tt + 1) * 128], in_=ps)
                        else:
                            nc.scalar.copy(
                                out=xnys[:, dd, tt * 128:(tt + 1) * 128], in_=ps)

            def in_proj_half(ns):
                # m-tiles 0..11 -> x (local order [own, peer]); 12..17 -> z own
                for m in range(18):
                    ps = psum.tile([128, 512], F32, tag="ps_mm")
                    for k in range(6):
                        nc.tensor.matmul(
                            ps, w_in_sb[k][:, m * 128:(m + 1) * 128],
                            xnys[:, k, ns * HL:(ns + 1) * HL],
                            start=(k == 0), stop=(k == 5))
                    if m < 12:
                        nc.scalar.activation(
                            out=xph[ns][:, m, 3:3 + HL], in_=ps,
                            func=AF.Identity, bias=biasin_t[:, m:m + 1])
                    else:
                        nc.scalar.activation(
                            out=sz[:, m - 12, ns * HL:(ns + 1) * HL], in_=ps,
                            func=AF.Silu, bias=biasin_t[:, m:m + 1])

            def conv_half(ns):
                # causal depthwise conv + silu, output in place over the input.
                # Edge cols 0:3 of half 1 are pre-conv copies from half 0.
                if ns == 0:
                    nc.vector.memset(xph[0][:, :, 0:3], 0.0)
                    nc.vector.tensor_copy(
                        out=xph[1][:, :, 0:3], in_=xph[0][:, :, HL:HL + 3])
                for g in range(12):
                    acc = rot.tile([128, HL], BF16, tag="conv_acc", bufs=2)
                    if CFG["conv_tsm"]:
                        # 4 tensor_scalar muls (4x DVE mode) + tree of adds
                        acb = rot.tile([128, HL], BF16, tag="conv_acb", bufs=2)
                        nc.vector.tensor_scalar_mul(acc, xph[ns][:, g, 0:HL], wconv_t[:, g, 0:1])
                        nc.vector.tensor_scalar_mul(acb, xph[ns][:, g, 1:1 + HL], wconv_t[:, g, 1:2])
                        nc.vector.tensor_add(acc, acc, acb)
                        nc.vector.tensor_scalar_mul(acb, xph[ns][:, g, 2:2 + HL], wconv_t[:, g, 2:3])
                        nc.vector.tensor_add(acc, acc, acb)
                        nc.vector.tensor_scalar_mul(acb, xph[ns][:, g, 3:3 + HL], wconv_t[:, g, 3:4])
                        nc.vector.tensor_add(acc, acc, acb)
                    else:
                        nc.vector.tensor_scalar_mul(acc, xph[ns][:, g, 0:HL], wconv_t[:, g, 0:1])
                        for k in range(1, 4):
                            nc.vector.scalar_tensor_tensor(
                                out=acc, in0=xph[ns][:, g, k:k + HL],
                                scalar=wconv_t[:, g, k:k + 1],
                                in1=acc, op0=OP.mult, op1=OP.add)
                    nc.scalar.activation(
                        out=xph[ns][:, g, 3:3 + HL], in_=acc, func=AF.Silu,
                        bias=bconv_t[:, g:g + 1])

            def xdbl_half(ns):
                ps = psum.tile([128, 512], F32, tag="ps_mm")
                for k in range(12):
                    nc.tensor.matmul(
                        ps[0:DTR + 2 * N, :], w_x_sb[k],
                        xph[ns][:, k, 3:3 + HL],
                        start=(k == 0), stop=(k == 11))
                nc.vector.tensor_copy(
                    out=bc_sb[:, ns * HL:(ns + 1) * HL], in_=ps[0:2 * N, :])
                nc.scalar.copy(
                    out=dt_t[0:32, ns * HL:(ns + 1) * HL], in_=ps[32:64, :])
                nc.scalar.copy(
                    out=dt_t[32:DTR, ns * HL:(ns + 1) * HL], in_=ps[64:2 * N + DTR, :])
                nc.sync.dma_start(
                    out=bc_bounce[:, ns * HL:(ns + 1) * HL],
                    in_=bc_sb[:, ns * HL:(ns + 1) * HL])

            def delta_half(ns):
                # delta = softplus(W_dt^T @ dt + b_dt): per-m Exp into delta,
                # then one fused Ln(1+e^x) in place (2 ACT table swaps total).
                pss = []
                for m in range(G):
                    ps = psum.tile([128, 512], F32, tag="ps_mm", name=f"psd{m}")
                    nc.tensor.matmul(
                        ps, w_dt_sb[:, m * 128:(m + 1) * 128],
                        dt_t[:, ns * HL:(ns + 1) * HL], start=True, stop=True)
                    pss.append(ps)
                for m in range(G):
                    nc.scalar.activation(
                        out=delta[:, m, ns * HL:(ns + 1) * HL], in_=pss[m],
                        func=AF.Exp, bias=bdt_t[:, m:m + 1])
                dsl = delta[:, :, ns * HL:(ns + 1) * HL]
                nc.scalar.activation(out=dsl, in_=dsl, func=AF.Ln, bias=1.0)

            def e_half(ns):
                # E = D*u*silu(z) into the dead peer half of xph[ns]
                for g in range(G):
                    nc.vector.scalar_tensor_tensor(
                        out=xph[ns][:, 6 + g, 3:3 + HL],
                        in0=xph[ns][:, g, 3:3 + HL], scalar=dpar_t[:, g:g + 1],
                        in1=sz[:, g, ns * HL:(ns + 1) * HL],
                        op0=OP.mult, op1=OP.mult)

            def epilogue(m, tail=False):
                # relu + residual + store for time tile m (after its AllReduce).
                # Tail epilogues (emitted after the chunk loop) run entirely on
                # the gpsimd/Pool queue: they wait on the last collectives, and
                # doing that on DVE/SP would head-block the next rep's work.
                tail = tail and CFG["tail_ep"] != "vector"
                eng = nc.gpsimd if tail else nc.vector
                dma = nc.gpsimd if (tail and CFG["tail_ep"] == "gpsimd") else nc.sync
                i = ch2b[m]
                s_sb = outp.tile([128, D], PDT, tag="s_sb")
                dma.dma_start(
                    out=s_sb,
                    in_=sum_ds[i][(m - bstart[i]) * TC:(m - bstart[i] + 1) * TC, :])
                xres = outp.tile([128, D], F32, tag="xres")
                dma.dma_start(out=xres, in_=x_in[m * 128:(m + 1) * 128, :])
                o2 = outp.tile([128, D], F32, tag="o2", bufs=2)
                eng.tensor_scalar_max(o2, s_sb, 0.0)
                eng.tensor_add(o2, o2, xres)
                dma.dma_start(out=out_d[m * 128:(m + 1) * 128, :], in_=o2)

            def gen_da(c):
                # dA[:, g, n, :] = exp(-(n+1) * delta) on the ACT engine.
                # Emitted right after scan(c-1) consumes da so the ACT work
                # overlaps the DVE readout of the previous chunk.
                d_sl = delta[:, :, c * TC:(c + 1) * TC]
                if CFG["da_pow"]:
                    for n in range(N):
                        nc.scalar.activation(
                            out=da[:, :, n, :], in_=d_sl,
                            func=AF.Exp, scale=-(n + 1.0))
                else:
                    nc.vector.tensor_mul(
                        da, d_sl[:, :, None, :].broadcast_to([128, G, N, TC]),
                        a_t[:, :, :, None].broadcast_to([128, G, N, TC]))
                    nc.scalar.activation(out=da_flat, in_=da_flat, func=AF.Exp)

            def chunk(c):
                csl = slice(c * TC, (c + 1) * TC)
                h, lo = c // (NCH // 2), (c % (NCH // 2)) * TC
                # one broadcast DMA for both B (rows 0:N) and C (rows N:2N)
                bcc = bccp.tile([128, 2 * N, TC], BF16, tag="bcc")
                nc.sync.dma_start(
                    out=bcc,
                    in_=ap_view(bc_bounce[:], c * TC, [[0, 128], [L, 2 * N], [1, TC]]))
                d_sl = delta[:, :, csl]
                u_sl = xph[h][:, 0:G, 3 + lo:3 + lo + TC]
                if not CFG["skip_gen"]:
                    # du = delta * u ; dBu[:, g, n, :] = du (bcast n) * B (bcast g)
                    du = rot.tile([128, G, TC], BF16, tag="du", bufs=2)
                    nc.vector.tensor_mul(du, d_sl, u_sl)
                    nc.vector.tensor_mul(
                        dbu, du[:, :, None, :].broadcast_to([128, G, N, TC]),
                        bcc[:, None, 0:N, :].broadcast_to([128, G, N, TC]))
                # fold carry into the t=0 column: h[n,0] = dA0*carry + dBu0
                if c > 0:
                    ctmp = rot.tile([128, G, N], BF16, tag="ctmp", bufs=2)
                    nc.vector.tensor_mul(ctmp, carry, da[:, :, :, 0])
                    nc.vector.tensor_add(dbu[:, :, :, 0], dbu[:, :, :, 0], ctmp)
                nc.vector.memset(da[:, :, :, 0:1], 0.0)
                # scan along flattened (g, n, t), in place over dBu; dA=0 at
                # segment starts so states never cross (g, n) boundaries.
                if not CFG["skip_scan"]:
                    nc.vector.tensor_tensor_scan(
                        out=dbu_flat, data0=da_flat, data1=dbu_flat,
                        initial=0.0, op0=OP.mult, op1=OP.add)
                # da is free after the scan: generate the next chunk's dA now
                # (chunk 3's successor needs delta half 1, emitted before it)
                if c + 1 < NCH and not CFG["skip_gen"]:
                    gen_da(c + 1)
                nc.vector.tensor_copy(out=carry, in_=dbu[:, :, :, TC - 1])
                ys_sl = ys[:, :, csl]
                if CFG["skip_readout"]:
                    nc.vector.tensor_copy(out=ys_sl, in_=dbu[:, :, 0, :])
                else:
                    # readout: hC in place -> tree-reduce over n -> ys
                    nc.vector.tensor_mul(
                        dbu, dbu, bcc[:, None, N:2 * N, :].broadcast_to([128, G, N, TC]))
                    nc.vector.tensor_add(dbu[:, :, 0:8, :], dbu[:, :, 0:8, :], dbu[:, :, 8:16, :])
                    nc.vector.tensor_add(dbu[:, :, 8:12, :], dbu[:, :, 0:4, :], dbu[:, :, 4:8, :])
                    nc.vector.tensor_add(dbu[:, :, 12:14, :], dbu[:, :, 8:10, :], dbu[:, :, 10:12, :])
                    nc.vector.tensor_add(ys_sl, dbu[:, :, 12, :], dbu[:, :, 13, :])
                    # gate: y = ys*silu(z) + D*u*silu(z)
                    nc.vector.tensor_mul(ys_sl, ys_sl, sz[:, :, csl])
                    nc.vector.tensor_add(
                        ys_sl, ys_sl, xph[h][:, 6:12, 3 + lo:3 + lo + TC])
                # out_proj m-tile for this chunk's time columns (m == c)
                if CFG["skip_op"]:
                    return
                m = c
                part = outp.tile([128, D], PDT, tag="part")
                for ns, nw in ((0, 512), (1, 256)):
                    ps = psum.tile([128, 512], F32, tag="ps_mm")
                    for k in range(6):
                        nc.tensor.matmul(
                            ps[:, 0:nw], ys[:, k, m * 128:(m + 1) * 128],
                            w_out_sb[k][:, ns * 512:ns * 512 + nw],
                            start=(k == 0), stop=(k == 5))
                    nc.scalar.copy(
                        out=part[:, ns * 512:ns * 512 + nw], in_=ps[:, 0:nw])
                if CFG["skip_coll"]:
                    return
                i = ch2b[m]
                nc.sync.dma_start(
                    out=part_ds[i][(m - bstart[i]) * TC:(m - bstart[i] + 1) * TC, :],
                    in_=part)
                # pair AllReduce per bucket; epilogues delayed so the
                # collective latency hides behind later scan work
                if m == bend[i] - 1:
                    nc.gpsimd.collective_compute(
                        "AllReduce", OP.add,
                        replica_groups=[[0, 1], [2, 3], [4, 5], [6, 7]],
                        ins=[part_ds[i][:]],
                        outs=[sum_ds[i][:]])
                lim = c + 1 - CFG["ep_delay"]
                allowed = sum(k for e, k in zip(bend, BK) if e <= lim)
                while ep_next[0] < min(allowed, NCH):
                    epilogue(ep_next[0])
                    ep_next[0] += 1

            # ------------- pipelined schedule: half-1 stages overlap -------
            # the scan of chunks 0..2 (PE/ACT work hides under DVE).
            # LN tiles 4-7 are deferred so in_proj half 0 (which only needs
            # xn cols 0:512 = tiles 0-3) starts as early as possible.
            layer_norm([0, 1, 2, 3])
            if rep == 0:
                load_weights()
            in_proj_half(0)
            conv_half(0)
            xdbl_half(0)
            if CFG["stop"] == "conv":
                layer_norm([4, 5, 6, 7])
                in_proj_half(1); conv_half(1)
                return
            delta_half(0)
            e_half(0)
            if CFG["stop"] == "xdbl":
                layer_norm([4, 5, 6, 7])
                in_proj_half(1); conv_half(1); xdbl_half(1)
                return
            if CFG["stop"] == "delta":
                layer_norm([4, 5, 6, 7])
                in_proj_half(1); conv_half(1); xdbl_half(1)
                delta_half(1); e_half(1)
                return
            if not CFG["skip_gen"]:
                gen_da(0)
            if not CFG["ln_late"]:
                layer_norm([4, 5, 6, 7])
            chunk(0)
            if CFG["ln_late"]:
                layer_norm([4, 5, 6, 7])
            in_proj_half(1)
            chunk(1)
            chunk(2)
            conv_half(1)
            xdbl_half(1)
            delta_half(1)
            e_half(1)
            for c in range(3, NCH):
                chunk(c)

            if CFG["stop"] in ("scan", "outproj") or CFG["skip_coll"] or CFG["skip_op"]:
                return
            # ---------------- remaining epilogues (Pool queue) ----------------
            while ep_next[0] < NCH:
                epilogue(ep_next[0], tail=True)
                ep_next[0] += 1

        for rep in range(repeat):
            body(rep)

    nc.compile()
    return nc


def make_in_maps(inputs):
    """Host-side sharding: per-core input dicts."""
    x = np.asarray(inputs["input_data"], np.float32)
    ln_g = np.asarray(inputs["ln_g"], np.float32)
    ln_b = np.asarray(inputs["ln_b"], np.float32)
    W_in = np.asarray(inputs["W_in"], np.float32)
    W_conv = np.asarray(inputs["W_conv"], np.float32)
    b_conv = np.asarray(inputs["b_conv"], np.float32)
    W_x = np.asarray(inputs["W_x"], np.float32)
    W_dt = np.asarray(inputs["W_dt"], np.float32)
    b_dt = np.asarray(inputs["b_dt"], np.float32)
    A_log = np.asarray(inputs["A_log"], np.float32)
    D_param = np.asarray(inputs["D_param"], np.float32)
    W_out = np.asarray(inputs["W_out"], np.float32)

    maps = []
    for c in range(8):
        b, h = c // 2, c % 2
        own = np.arange(h * DH, (h + 1) * DH)
        peer = np.arange((1 - h) * DH, (2 - h) * DH)
        perm = np.concatenate([own, peer])
        W_sel = np.concatenate([W_in[perm], W_in[DI + own]], 0)  # (2304, 768)
        maps.append({
            "x_in": np.ascontiguousarray(x[b]),
            "w_in_t": np.ascontiguousarray((W_sel * ln_g[None, :]).T).astype(NPBF16),
            "bias_in": np.ascontiguousarray(W_sel @ ln_b),
            "w_conv": np.ascontiguousarray(W_conv[perm, 0, :]),
            "b_conv": np.ascontiguousarray(b_conv[perm]),
            "w_x_t": np.ascontiguousarray(W_x[np.r_[DTR:DTR + 2 * N, 0:DTR]][:, perm].T).astype(NPBF16),
            "w_dt_t": np.ascontiguousarray(W_dt[own].T).astype(NPBF16),
            "b_dt": np.ascontiguousarray(b_dt[own]),
            "a_neg": np.ascontiguousarray(-np.exp(A_log[own])).astype(NPBF16),
            "d_par": np.ascontiguousarray(D_param[own]),
            "w_out_t": np.ascontiguousarray(W_out[:, own].T).astype(NPBF16),
        })
    return maps


_CACHED = {}


def kernel(**inputs) -> np.ndarray:
    # The fast dA path exploits A = -exp(A_log) = -[1..N] broadcast over
    # channels (dA_n = r^(n+1), r = exp(-delta)). Verify and fall back to
    # the general elementwise-exp path if A_log doesn't match that pattern.
    a_log = np.asarray(inputs["A_log"], np.float32)
    geo = np.log(np.broadcast_to(np.arange(1, N + 1, dtype=np.float32), (DI, N)))
    da_pow = bool(np.allclose(a_log, geo, rtol=1e-6, atol=1e-6))
    if da_pow not in _CACHED:
        _CACHED[da_pow] = build_program({"da_pow": da_pow})
    nc = _CACHED[da_pow]
    maps = make_in_maps(inputs)
    res = run_bass_kernel_spmd(nc, maps, core_ids=list(range(8)))
    out = np.stack([res.results[2 * b]["out"] for b in range(B)], 0)
    return out.astype(np.float32)


if __name__ == "__main__":
    rng = np.random.default_rng(0)
    ins = {
        "input_data": rng.standard_normal((B, L, D)).astype(np.float32),
        "ln_g": np.ones((D,), np.float32),
        "ln_b": np.zeros((D,), np.float32),
        "W_in": (rng.standard_normal((2 * DI, D)) * 0.02).astype(np.float32),
        "W_conv": (rng.standard_normal((DI, 1, DCONV)) * 0.02).astype(np.float32),
        "b_conv": np.zeros((DI,), np.float32),
        "W_x": (rng.standard_normal((DTR + 2 * N, DI)) * 0.02).astype(np.float32),
        "W_dt": (rng.standard_normal((DI, DTR)) * 0.02).astype(np.float32),
        "b_dt": (rng.standard_normal((DI,)) * 0.1).astype(np.float32),
        "A_log": np.log(np.broadcast_to(np.arange(1, N + 1, dtype=np.float32), (DI, N))).copy(),
        "D_param": np.ones((DI,), np.float32),
        "W_out": (rng.standard_normal((D, DI)) * 0.02).astype(np.float32),
    }
    out = kernel(**ins)
    print("kernel out", out.shape, out.dtype)
